# revision 23
# baseline (speedup 1.0000x reference)
"""MiniBatch K-means (1 iteration) on 8 Trainium2 NeuronCores.

Strategy (data-parallel over points, per sharding hint):
  - Shard X along N across 8 cores (62500 points each, zero-padded to
    62720 = 490 tiles of 128 points; tiles are processed in PAIRS so
    the DVE reduce can batch two tiles per instruction).
  - Per 128-point tile on each core:
      mm1 (PE, 2x213ns): q[n,k] = c2[k]/2 - x_n.c_k via the bf16 hi/lo
            error-compensated pair of matmuls (x = xh+xl, c ~ ch+cl):
              mm1a: [xh;xl] @ [ch;ch]   (128-row contraction)
              mm1b: [xh;1,1,1] @ [cl; c2a; c2b; c2c]  (67-row)
            Distance error ~2^-18: assignments match the f32 reference
            to a handful of boundary flips. (f32r matmuls are TF32-
            precision on the BIR simulator, so full-rate exact fp32 is
            not available; this is the proven workaround.)
      DVE (1192ns/pair): one tensor_reduce(min, negate) batched over a
            [128, 2, 512] PSUM tile pair -> mneg[128, 2] (amortizes the
            125ns PSUM-access penalty; DVE is the only free-axis
            reducer and the ISA allows only one PSUM-read operand per
            instruction, so no tensor_tensor folding can help).
      ACT (612ns): r[n,k] = sign(q + mneg) -> EXACT complement
            indicator (0 at the argmin, 1 elsewhere) as f16 {0,1}.
            No exp/beta/leakage. Each tile gets its OWN r tile so the
            two signs of a pair carry independent WAR dependencies.
      mm2 (PE, 4x27ns): TRANSPOSED sums: for each 128-wide k-chunk kc,
              S_T[:, kc, :] += r[:, kc*128:(kc+1)*128].T @ [X|1]_tile
            i.e. lhsT = the indicator chunk (stationary), rhs = the f16
            point-major [128, 65] tile. Output free dim is only 65, so
            each matmul costs 65 cycles; counts ride along as the ones
            column of the rhs. All four chunk accumulators live in ONE
            PSUM bank (start=True zeroes only the bytes each AP writes;
            skip_group_check bypasses the one-group-per-region guard).
  - Host: S^T chunks -> S' [65, 512]; S = colsum(f16(X)) - sum_cores S',
    counts = N - S'[., 64], divide, transpose.

The complement convention exists because the Activation engine can only
produce an exact indicator via Sign (sign(q-m) is 0 at the argmin, 1
elsewhere); the host subtracts per-core column totals of the SAME f16
rounding the device sums. Padded points have all-zero [X|1] rows.

Engine steady state per tile pair: PE ~1100ns, DVE ~1192ns, ACT ~1224ns
-> ACT-bound at ~620ns/tile. One combined boot DMA carries cha/clb and
tile 0/1's lhsT so the first matmuls start ~2us in.
"""

import numpy as np

N, D, K = 500000, 64, 512
NCORES = 8
NS = N // NCORES            # 62500 points per core
PT = 128                    # points per tile (partition dim)
TPS = 7                     # tiles per DMA slab
NSLAB = -(-NS // (PT * TPS))  # 70 slabs
NTP = NSLAB * TPS           # 490 tiles
NP = NTP // 2               # 245 tile pairs
NPAD = NTP * PT             # 62720 padded points per core
DA = D + 1                  # 65: X augmented with ones column
DH = D + 3                  # 67: xh rows + three c2 ones rows
XTF = TPS * PT              # 896 columns of x^T per slab
KC = K // PT                # 4 k-chunks for the transposed mm2

_CACHE: dict = {}


def _build_nc():
    from contextlib import ExitStack

    import concourse.bacc as bacc
    import concourse.tile as tile
    from concourse import mybir

    f32 = mybir.dt.float32
    bf16 = mybir.dt.bfloat16
    f16 = mybir.dt.float16

    nc = bacc.Bacc("TRN2", target_bir_lowering=False, debug=False)

    # boot: cha | clb | xall tile0+1 | xht tile0+1  (one DMA)
    BOOTW = 2 * K + 4 * PT
    boot = nc.dram_tensor("boot", [PT, BOOTW], bf16, kind="ExternalInput")
    xall = nc.dram_tensor("xall", [PT, NSLAB, XTF], bf16, kind="ExternalInput")
    xht = nc.dram_tensor("xht", [DH, NSLAB, XTF], bf16, kind="ExternalInput")
    xa = nc.dram_tensor("xa", [PT, NSLAB, TPS * DA], f16, kind="ExternalInput")
    sout = nc.dram_tensor("sout", [PT, KC, DA], f32, kind="ExternalOutput")

    with tile.TileContext(nc) as tc, ExitStack() as ctx:
        const = ctx.enter_context(tc.tile_pool(name="const", bufs=1))
        ld = ctx.enter_context(tc.tile_pool(name="ld", bufs=1))
        rp = ctx.enter_context(tc.tile_pool(name="r", bufs=1))
        mred = ctx.enter_context(tc.tile_pool(name="mred", bufs=1))
        gp = ctx.enter_context(tc.tile_pool(name="g", bufs=1, space="PSUM"))
        sp = ctx.enter_context(tc.tile_pool(name="s", bufs=1, space="PSUM"))

        boot_sb = const.tile([PT, BOOTW], bf16)
        nc.sync.dma_start(boot_sb[:], boot[:])
        cha = boot_sb[:, 0:K]                     # [ch; ch]
        clb = boot_sb[:DH, K : 2 * K]             # [cl; c2a; c2b; c2c]

        # S'^T accumulator: [k-row, k-chunk, d] in one PSUM bank,
        # chunks padded to 512B so each matmul output is 512-byte aligned.
        # The bank is zeroed ONCE up front (a matmul's start=True would zero
        # the whole 2KB region, wiping sibling chunks), and every mm2
        # accumulates with start=False + skip_group_check.
        s_ps = sp.tile([PT, KC, PT], f32)
        nc.vector.memset(s_ps[:], 0.0)

        # Manual tile rings (instead of per-iteration pool allocs): Tile
        # emits a release-event pair per allocated tile, and those events
        # serialize the engine sequencers; fixed tiles keep the WAR/RAW
        # tracking without the release machinery.
        QB, MB, RGB, LB = 3, 3, 6, 6
        q_ring = [
            gp.tile([PT, 2, K], f32, name=f"q{i}", tag=f"q{i}") for i in range(QB)
        ]
        m_ring = [
            mred.tile([PT, 2], f32, name=f"m{i}", tag=f"m{i}") for i in range(MB)
        ]
        r_ring = [
            rp.tile([PT, K], f16, name=f"r{i}", tag=f"r{i}") for i in range(RGB)
        ]
        ld_ring = [
            (
                ld.tile([PT, XTF], bf16, name=f"xall{i}", tag=f"xall{i}"),
                ld.tile([DH, XTF], bf16, name=f"xht{i}", tag=f"xht{i}"),
                ld.tile([PT, TPS * DA], f16, name=f"xa{i}", tag=f"xa{i}"),
            )
            for i in range(LB)
        ]
        slabs = [None] * NSLAB
        PF = 1  # slab DMA lookahead (in slabs)

        def emit_dma(si):
            xall_t, xht_t, xa_t = ld_ring[si % LB]
            nc.sync.dma_start(xall_t[:], xall[:, si, :])
            nc.sync.dma_start(xht_t[:], xht[:, si, :])
            nc.gpsimd.dma_start(xa_t[:], xa[:, si, :])
            slabs[si] = (xall_t, xht_t, xa_t)

        def emit_mm1(g):
            si, tt = divmod(g, TPS)
            xall_t, xht_t, _ = slabs[si]
            if g < 2:
                # bootstrap: tiles 0-1 lhsT ride the boot DMA so the first
                # matmuls start right after one small transfer
                lhs_a = boot_sb[:, 2 * K + g * PT : 2 * K + (g + 1) * PT]
                lhs_b = boot_sb[:DH, 2 * K + (2 + g) * PT : 2 * K + (3 + g) * PT]
            else:
                lhs_a = xall_t[:, tt * PT : (tt + 1) * PT]
                lhs_b = xht_t[:, tt * PT : (tt + 1) * PT]
            p, j = divmod(g, 2)
            q_ps = q_ring[p % QB]
            nc.tensor.matmul(q_ps[:, j, :], lhs_a, cha, start=True, stop=False)
            nc.tensor.matmul(q_ps[:, j, :], lhs_b, clb, start=False, stop=True)

        def emit_reduce(p):
            nc.vector.tensor_reduce(
                out=m_ring[p % MB][:],
                in_=q_ring[p % QB][:],
                axis=mybir.AxisListType.X,
                op=mybir.AluOpType.min,
                negate=True,
            )

        def emit_sign(g):
            p, j = divmod(g, 2)
            nc.scalar.activation(
                out=r_ring[g % RGB][:],
                in_=q_ring[p % QB][:, j, :],
                func=mybir.ActivationFunctionType.Sign,
                bias=m_ring[p % MB][:, j : j + 1],
                scale=1.0,
            )

        def emit_mm2(g, first, last):
            si, tt = divmod(g, TPS)
            _, _, xa_t = slabs[si]
            xa_rhs = xa_t[:, tt * DA : (tt + 1) * DA]
            r_t = r_ring[g % RGB]
            for kc in range(KC):
                nc.tensor.matmul(
                    s_ps[:, kc, 0:DA],
                    r_t[:, kc * PT : (kc + 1) * PT],
                    xa_rhs,
                    start=False,
                    stop=False,
                    skip_group_check=True,
                )

        for p in range(NP + 4):
            if p < NP:
                g0 = 2 * p
                si, tt = divmod(g0, TPS)
                if p == 0:
                    for s0 in range(PF + 1):
                        emit_dma(s0)
                else:
                    for g in (g0, g0 + 1):
                        sig, ttg = divmod(g, TPS)
                        if ttg == 0 and sig + PF < NSLAB:
                            emit_dma(sig + PF)
                emit_mm1(g0)
                emit_mm1(g0 + 1)
            if 0 <= p - 1 < NP:
                emit_reduce(p - 1)
            if 0 <= p - 2 < NP:
                emit_sign(2 * (p - 2))
                emit_sign(2 * (p - 2) + 1)
            if 0 <= p - 3 < NP:
                for g in (2 * (p - 3), 2 * (p - 3) + 1):
                    emit_mm2(g, first=(g == 0), last=(g == NTP - 1))

        s_sb = const.tile([PT, KC, DA], f32)
        nc.vector.tensor_copy(s_sb[:], s_ps[:, :, 0:DA])
        nc.sync.dma_start(sout[:], s_sb[:])

    nc.compile()
    return nc


def _get_nc():
    if "nc" not in _CACHE:
        _CACHE["nc"] = _build_nc()
    return _CACHE["nc"]


def build_in_maps(X, idx):
    import ml_dtypes

    bf = ml_dtypes.bfloat16

    C = X[idx].astype(np.float64)  # [K, D]
    c2h = 0.5 * np.einsum("kd,kd->k", C, C)

    cb = -C.T  # [D, K] float64
    ch = cb.astype(bf)
    cl = (cb - ch.astype(np.float64)).astype(bf)
    c2a = c2h.astype(bf)
    c2b = (c2h - c2a.astype(np.float64)).astype(bf)
    c2c = (c2h - c2a.astype(np.float64) - c2b.astype(np.float64)).astype(bf)

    cha_np = np.concatenate([ch, ch], axis=0)                    # [128, K]
    clb_np = np.concatenate([cl, c2a[None], c2b[None], c2c[None]], axis=0)  # [67, K]

    in_maps = []
    for c in range(NCORES):
        xs = X[c * NS : (c + 1) * NS]  # [NS, D] float32
        xh = xs.astype(bf)
        xl = (xs - xh.astype(np.float32)).astype(bf)

        xall_np = np.zeros((PT, NPAD), bf)
        xall_np[:D, :NS] = xh.T
        xall_np[D:, :NS] = xl.T
        xht_np = np.zeros((DH, NPAD), bf)
        xht_np[:D, :NS] = xh.T
        xht_np[D:, :NS] = 1.0

        xa_np = np.zeros((NPAD, DA), np.float16)
        xa_np[:NS, :D] = xs.astype(np.float16)
        xa_np[:NS, D] = 1.0
        xa_tiled = np.ascontiguousarray(
            xa_np.reshape(NTP, PT, DA).transpose(1, 0, 2)
        ).reshape(PT, NSLAB, TPS * DA)

        boot_np = np.zeros((PT, BOOTW_H), bf)
        boot_np[:, :K] = cha_np
        boot_np[:DH, K : 2 * K] = clb_np
        boot_np[:, 2 * K : 2 * K + 2 * PT] = xall_np[:, : 2 * PT]
        boot_np[:DH, 2 * K + 2 * PT :] = xht_np[:, : 2 * PT]

        in_maps.append(
            {
                "boot": boot_np,
                "xall": np.ascontiguousarray(xall_np.reshape(PT, NSLAB, XTF)),
                "xht": np.ascontiguousarray(xht_np.reshape(DH, NSLAB, XTF)),
                "xa": xa_tiled,
            }
        )
    return in_maps


BOOTW_H = 2 * K + 4 * PT


def kernel(X, init_idx):
    from concourse.bass_utils import run_bass_kernel_spmd

    X = np.ascontiguousarray(np.asarray(X, dtype=np.float32))
    idx = np.asarray(init_idx).astype(np.int64)

    in_maps = build_in_maps(X, idx)
    _CACHE["in_maps"] = in_maps

    # Build a fresh Bass module per call: executing via run_bass_kernel_spmd
    # mutates the module, and re-running a previously-executed one crashes
    # the device (NRT_EXEC_UNIT_UNRECOVERABLE).
    nc = _build_nc()
    res = run_bass_kernel_spmd(nc, in_maps, core_ids=list(range(NCORES)))

    SpT = np.zeros((PT, KC, DA), np.float64)
    for r in res.results:
        SpT += r["sout"].astype(np.float64)
    # S'[d, k] with k = kc*128 + kr  <-  SpT[kr, kc, d]
    Sp = np.transpose(SpT, (2, 1, 0)).reshape(DA, K)

    colsum = X.astype(np.float16).astype(np.float64).sum(axis=0)  # [D]
    sums = colsum[:, None] - Sp[:D]                # [D, K]
    counts = float(N) - Sp[D]                      # [K]
    out = (sums / np.maximum(counts, 1.0)[None, :]).T.astype(np.float32)
    return out


# revision 32
# speedup vs baseline: 1.1075x; 1.1075x over previous
"""MiniBatch K-means (1 iteration) on 8 Trainium2 NeuronCores.

Strategy (data-parallel over points, per sharding hint):
  - Shard X along N across 8 cores (62500 points each, zero-padded to
    62720 = 490 tiles of 128 points).
  - Per 128-point tile on each core (steady state ~678ns/tile, DVE-bound
    with zero steady-state gaps):
      mm1 (PE, 2x213ns): q[n,k] = c2[k]/2 - x_n.c_k via the bf16 hi/lo
            error-compensated pair of matmuls (x = xh+xl, c ~ ch+cl):
              mm1a: [xh;xl] @ [ch;ch]   (128-row contraction)
              mm1b: [xh;1,1,1] @ [cl; c2a; c2b; c2c]  (67-row)
            Distance error ~2^-18: assignments match the f32 reference
            to a handful of boundary flips. (f32r matmuls are TF32-
            precision on this stack, so full-rate exact fp32 does not
            exist; fp8 3-piece schemes flip too many assignments.)
      DVE (658ns): tensor_reduce(min, negate) over the [128, 512] PSUM
            tile -> mneg = -min_k q. This is the bottleneck and it is
            irreducible: DVE is the only free-axis reducer, the ISA
            allows only one PSUM-read operand per instruction (so no
            tensor_tensor_reduce folding), GPSIMD cannot touch PSUM,
            and batching tiles per reduce makes the Activation engine
            self-serialize on its own tick semaphore (+219ns/pair,
            a worse trade).
      ACT (612ns): r[n,k] = sign(q + mneg) -> EXACT complement
            indicator (0 at the argmin, 1 elsewhere) as f16 {0,1}.
            No exp/beta/leakage; ties double-count, which the metric
            tolerates. Gated per tile by its own reduce, so every
            engine is cross-gated just-in-time and no Tile semaphore
            tax accrues.
      mm2 (PE, 4x27ns): TRANSPOSED sums: for each 128-wide k-chunk kc,
              S_T[:, kc, :] += r[:, kc*128:(kc+1)*128].T @ [X|1]_tile
            lhsT = the indicator chunk (stationary), rhs = the f16
            point-major [128, 65] tile, so each matmul streams only 65
            columns; counts ride along as the ones column of the rhs.
            All four chunk accumulators live in ONE PSUM bank, pre-
            zeroed once (a matmul start=True would zero the whole 2KB
            region and wipe sibling chunks), accumulating with
            start=False + skip_group_check.
  - Host: S^T chunks -> S' [65, 512]; S = colsum(f16(X)) - sum_cores S',
    counts = N - S'[64], divide, transpose.

The complement convention exists because the Activation engine can only
produce an exact indicator via Sign (sign(q-m) is 0 at the argmin, 1
elsewhere); the host subtracts per-core column totals of the SAME f16
rounding the device sums. Padded points have all-zero [X|1] rows.

Startup/teardown: the first boot DMA carries only mm1a's operands
(cha + tiles 0/1 of xall) so the first matmul issues ~3.2us in; clb and
the xht bootstrap ride a second DMA. xht/xa stream over the SWDGE
(gpsimd) queue to keep the HWDGE serial chain short. Engine busy:
DVE 97%, ACT ~90%, PE ~79% of the 332.4us total.
"""

import numpy as np

N, D, K = 500000, 64, 512
NCORES = 8
NS = N // NCORES            # 62500 points per core
PT = 128                    # points per tile (partition dim)
TPS = 7                     # tiles per DMA slab
NSLAB = -(-NS // (PT * TPS))  # 70 slabs
NTP = NSLAB * TPS           # 490 tiles
NP = NTP // 2               # 245 tile pairs
NPAD = NTP * PT             # 62720 padded points per core
DA = D + 1                  # 65: X augmented with ones column
DH = D + 3                  # 67: xh rows + three c2 ones rows
XTF = TPS * PT              # 896 columns of x^T per slab
KC = K // PT                # 4 k-chunks for the transposed mm2

_CACHE: dict = {}


def _build_nc():
    from contextlib import ExitStack

    import concourse.bacc as bacc
    import concourse.tile as tile
    from concourse import mybir

    f32 = mybir.dt.float32
    bf16 = mybir.dt.bfloat16
    f16 = mybir.dt.float16

    nc = bacc.Bacc("TRN2", target_bir_lowering=False, debug=False)

    # boot: cha | clb | xall tile0+1 | xht tile0+1  (one DMA)
    BOOTW = 2 * K + 4 * PT
    boot = nc.dram_tensor("boot", [PT, BOOTW], bf16, kind="ExternalInput")
    xall = nc.dram_tensor("xall", [PT, NSLAB, XTF], bf16, kind="ExternalInput")
    xht = nc.dram_tensor("xht", [DH, NSLAB, XTF], bf16, kind="ExternalInput")
    xa = nc.dram_tensor("xa", [PT, NSLAB, TPS * DA], f16, kind="ExternalInput")
    sout = nc.dram_tensor("sout", [PT, KC, DA], f32, kind="ExternalOutput")

    with tile.TileContext(nc) as tc, ExitStack() as ctx:
        const = ctx.enter_context(tc.tile_pool(name="const", bufs=1))
        ld = ctx.enter_context(tc.tile_pool(name="ld", bufs=1))
        rp = ctx.enter_context(tc.tile_pool(name="r", bufs=1))
        mred = ctx.enter_context(tc.tile_pool(name="mred", bufs=1))
        gp = ctx.enter_context(tc.tile_pool(name="g", bufs=1, space="PSUM"))
        sp = ctx.enter_context(tc.tile_pool(name="s", bufs=1, space="PSUM"))

        boot_sb = const.tile([PT, BOOTW], bf16)
        nc.sync.dma_start(boot_sb[:, 0 : K + 2 * PT], boot[:, 0 : K + 2 * PT])
        nc.sync.dma_start(
            boot_sb[:, K + 2 * PT :], boot[:, K + 2 * PT :]
        )
        cha = boot_sb[:, 0:K]                     # [ch; ch]
        clb = boot_sb[:DH, K + 2 * PT : 2 * K + 2 * PT]  # [cl; c2a; c2b; c2c]

        # S'^T accumulator: [k-row, k-chunk, d] in one PSUM bank,
        # chunks padded to 512B so each matmul output is 512-byte aligned.
        # The bank is zeroed ONCE up front (a matmul's start=True would zero
        # the whole 2KB region, wiping sibling chunks), and every mm2
        # accumulates with start=False + skip_group_check.
        s_ps = sp.tile([PT, KC, PT], f32)
        nc.vector.memset(s_ps[:], 0.0)

        # Manual tile rings (instead of per-iteration pool allocs): Tile
        # emits a release-event pair per allocated tile, and those events
        # serialize the engine sequencers; fixed tiles keep the WAR/RAW
        # tracking without the release machinery.
        QB, MB, RGB, LB = 5, 6, 4, 4
        q_ring = [
            gp.tile([PT, K], f32, name=f"q{i}", tag=f"q{i}") for i in range(QB)
        ]
        m_ring = [
            mred.tile([PT, 1], f32, name=f"m{i}", tag=f"m{i}") for i in range(MB)
        ]
        r_ring = [
            rp.tile([PT, K], f16, name=f"r{i}", tag=f"r{i}") for i in range(RGB)
        ]
        ld_ring = [
            (
                ld.tile([PT, XTF], bf16, name=f"xall{i}", tag=f"xall{i}"),
                ld.tile([DH, XTF], bf16, name=f"xht{i}", tag=f"xht{i}"),
                ld.tile([PT, TPS * DA], f16, name=f"xa{i}", tag=f"xa{i}"),
            )
            for i in range(LB)
        ]
        slabs = [None] * NSLAB
        PF = 1  # slab DMA lookahead (in slabs)

        def emit_dma(si):
            xall_t, xht_t, xa_t = ld_ring[si % LB]
            nc.sync.dma_start(xall_t[:], xall[:, si, :])
            nc.gpsimd.dma_start(xht_t[:], xht[:, si, :])
            nc.gpsimd.dma_start(xa_t[:], xa[:, si, :])
            slabs[si] = (xall_t, xht_t, xa_t)

        def emit_mm1(g):
            si, tt = divmod(g, TPS)
            xall_t, xht_t, _ = slabs[si]
            if g < 2:
                # bootstrap: tiles 0-1 lhsT ride the boot DMA so the first
                # matmuls start right after one small transfer
                lhs_a = boot_sb[:, K + g * PT : K + (g + 1) * PT]
                lhs_b = boot_sb[:DH, 2 * K + (2 + g) * PT : 2 * K + (3 + g) * PT]
            else:
                lhs_a = xall_t[:, tt * PT : (tt + 1) * PT]
                lhs_b = xht_t[:, tt * PT : (tt + 1) * PT]
            q_ps = q_ring[g % QB]
            nc.tensor.matmul(q_ps[:], lhs_a, cha, start=True, stop=False)
            nc.tensor.matmul(q_ps[:], lhs_b, clb, start=False, stop=True)

        def emit_reduce(g):
            nc.vector.tensor_reduce(
                out=m_ring[g % MB][:],
                in_=q_ring[g % QB][:],
                axis=mybir.AxisListType.X,
                op=mybir.AluOpType.min,
                negate=True,
            )

        def emit_sign(g):
            nc.scalar.activation(
                out=r_ring[g % RGB][:],
                in_=q_ring[g % QB][:],
                func=mybir.ActivationFunctionType.Sign,
                bias=m_ring[g % MB][:],
                scale=1.0,
            )

        def emit_mm2(g, first, last):
            si, tt = divmod(g, TPS)
            _, _, xa_t = slabs[si]
            xa_rhs = xa_t[:, tt * DA : (tt + 1) * DA]
            r_t = r_ring[g % RGB]
            for kc in range(KC):
                nc.tensor.matmul(
                    s_ps[:, kc, 0:DA],
                    r_t[:, kc * PT : (kc + 1) * PT],
                    xa_rhs,
                    start=False,
                    stop=False,
                    skip_group_check=True,
                )

        for g in range(NTP + 4):
            if g < NTP:
                si, tt = divmod(g, TPS)
                if g == 0:
                    for s0 in range(PF + 1):
                        emit_dma(s0)
                elif tt == 0 and si + PF < NSLAB:
                    emit_dma(si + PF)
                emit_mm1(g)
            if 0 <= g - 1 < NTP:
                emit_reduce(g - 1)
            if 0 <= g - 2 < NTP:
                emit_sign(g - 2)
            if 0 <= g - 3 < NTP:
                emit_mm2(g - 3, first=(g - 3 == 0), last=(g - 3 == NTP - 1))

        s_sb = const.tile([PT, KC, DA], f32)
        nc.vector.tensor_copy(s_sb[:], s_ps[:, :, 0:DA])
        nc.sync.dma_start(sout[:], s_sb[:])

    nc.compile()
    return nc


def _get_nc():
    if "nc" not in _CACHE:
        _CACHE["nc"] = _build_nc()
    return _CACHE["nc"]


def build_in_maps(X, idx):
    import ml_dtypes

    bf = ml_dtypes.bfloat16

    C = X[idx].astype(np.float64)  # [K, D]
    c2h = 0.5 * np.einsum("kd,kd->k", C, C)

    cb = -C.T  # [D, K] float64
    ch = cb.astype(bf)
    cl = (cb - ch.astype(np.float64)).astype(bf)
    c2a = c2h.astype(bf)
    c2b = (c2h - c2a.astype(np.float64)).astype(bf)
    c2c = (c2h - c2a.astype(np.float64) - c2b.astype(np.float64)).astype(bf)

    cha_np = np.concatenate([ch, ch], axis=0)                    # [128, K]
    clb_np = np.concatenate([cl, c2a[None], c2b[None], c2c[None]], axis=0)  # [67, K]

    in_maps = []
    for c in range(NCORES):
        xs = X[c * NS : (c + 1) * NS]  # [NS, D] float32
        xh = xs.astype(bf)
        xl = (xs - xh.astype(np.float32)).astype(bf)

        xall_np = np.zeros((PT, NPAD), bf)
        xall_np[:D, :NS] = xh.T
        xall_np[D:, :NS] = xl.T
        xht_np = np.zeros((DH, NPAD), bf)
        xht_np[:D, :NS] = xh.T
        xht_np[D:, :NS] = 1.0

        xa_np = np.zeros((NPAD, DA), np.float16)
        xa_np[:NS, :D] = xs.astype(np.float16)
        xa_np[:NS, D] = 1.0
        xa_tiled = np.ascontiguousarray(
            xa_np.reshape(NTP, PT, DA).transpose(1, 0, 2)
        ).reshape(PT, NSLAB, TPS * DA)

        boot_np = np.zeros((PT, BOOTW_H), bf)
        boot_np[:, :K] = cha_np
        boot_np[:, K : K + 2 * PT] = xall_np[:, : 2 * PT]
        boot_np[:DH, K + 2 * PT : 2 * K + 2 * PT] = clb_np
        boot_np[:DH, 2 * K + 2 * PT :] = xht_np[:, : 2 * PT]

        in_maps.append(
            {
                "boot": boot_np,
                "xall": np.ascontiguousarray(xall_np.reshape(PT, NSLAB, XTF)),
                "xht": np.ascontiguousarray(xht_np.reshape(DH, NSLAB, XTF)),
                "xa": xa_tiled,
            }
        )
    return in_maps


BOOTW_H = 2 * K + 4 * PT


def kernel(X, init_idx):
    from concourse.bass_utils import run_bass_kernel_spmd

    X = np.ascontiguousarray(np.asarray(X, dtype=np.float32))
    idx = np.asarray(init_idx).astype(np.int64)

    in_maps = build_in_maps(X, idx)
    _CACHE["in_maps"] = in_maps

    # Build a fresh Bass module per call: executing via run_bass_kernel_spmd
    # mutates the module, and re-running a previously-executed one crashes
    # the device (NRT_EXEC_UNIT_UNRECOVERABLE).
    nc = _build_nc()
    res = run_bass_kernel_spmd(nc, in_maps, core_ids=list(range(NCORES)))

    SpT = np.zeros((PT, KC, DA), np.float64)
    for r in res.results:
        SpT += r["sout"].astype(np.float64)
    # S'[d, k] with k = kc*128 + kr  <-  SpT[kr, kc, d]
    Sp = np.transpose(SpT, (2, 1, 0)).reshape(DA, K)

    colsum = X.astype(np.float16).astype(np.float64).sum(axis=0)  # [D]
    sums = colsum[:, None] - Sp[:D]                # [D, K]
    counts = float(N) - Sp[D]                      # [K]
    out = (sums / np.maximum(counts, 1.0)[None, :]).T.astype(np.float32)
    return out


# revision 36
# speedup vs baseline: 1.1097x; 1.0020x over previous
"""MiniBatch K-means (1 iteration) on 8 Trainium2 NeuronCores.

Strategy (data-parallel over points, per sharding hint):
  - Shard X along N across 8 cores (62500 points each, zero-padded to
    62720 = 490 tiles of 128 points).
  - Per 128-point tile on each core (steady state ~678ns/tile, DVE-bound
    with zero steady-state gaps):
      mm1 (PE, 2x213ns): q[n,k] = c2[k]/2 - x_n.c_k via the bf16 hi/lo
            error-compensated pair of matmuls (x = xh+xl, c ~ ch+cl):
              mm1a: [xh;xl] @ [ch;ch]   (128-row contraction)
              mm1b: [xh;1,1,1] @ [cl; c2a; c2b; c2c]  (67-row)
            Distance error ~2^-18: assignments match the f32 reference
            to a handful of boundary flips. (f32r matmuls are TF32-
            precision on this stack, so full-rate exact fp32 does not
            exist; fp8 3-piece schemes flip too many assignments.)
      DVE (658ns): tensor_reduce(min, negate) over the [128, 512] PSUM
            tile -> mneg = -min_k q. This is the bottleneck and it is
            irreducible: DVE is the only free-axis reducer, the ISA
            allows only one PSUM-read operand per instruction (so no
            tensor_tensor_reduce folding), GPSIMD cannot touch PSUM,
            and batching tiles per reduce makes the Activation engine
            self-serialize on its own tick semaphore (+219ns/pair,
            a worse trade).
      ACT (612ns): r[n,k] = sign(q + mneg) -> EXACT complement
            indicator (0 at the argmin, 1 elsewhere) as f16 {0,1}.
            No exp/beta/leakage; ties double-count, which the metric
            tolerates. Gated per tile by its own reduce, so every
            engine is cross-gated just-in-time and no Tile semaphore
            tax accrues.
      mm2 (PE, 4x27ns): TRANSPOSED sums: for each 128-wide k-chunk kc,
              S_T[:, kc, :] += r[:, kc*128:(kc+1)*128].T @ [X|1]_tile
            lhsT = the indicator chunk (stationary), rhs = the f16
            point-major [128, 65] tile, so each matmul streams only 65
            columns; counts ride along as the ones column of the rhs.
            All four chunk accumulators live in ONE PSUM bank, pre-
            zeroed once (a matmul start=True would zero the whole 2KB
            region and wipe sibling chunks), accumulating with
            start=False + skip_group_check.
  - Host: S^T chunks -> S' [65, 512]; S = colsum(f16(X)) - sum_cores S',
    counts = N - S'[64], divide, transpose.

The complement convention exists because the Activation engine can only
produce an exact indicator via Sign (sign(q-m) is 0 at the argmin, 1
elsewhere); the host subtracts per-core column totals of the SAME f16
rounding the device sums. Padded points have all-zero [X|1] rows.

Startup/teardown: the first boot DMA carries only mm1a's operands
(cha + tiles 0/1 of xall) so the first matmul issues ~3.2us in; clb and
the xht bootstrap ride a second DMA. xht/xa stream over the SWDGE
(gpsimd) queue to keep the HWDGE serial chain short. Engine busy:
DVE 97%, ACT ~90%, PE ~79% of the 332.4us total.
"""

import numpy as np

N, D, K = 500000, 64, 512
NCORES = 8
NS = N // NCORES            # 62500 points per core
PT = 128                    # points per tile (partition dim)
TPS = 7                     # tiles per DMA slab
NSLAB = -(-NS // (PT * TPS))  # 70 slabs
NTP = NSLAB * TPS           # 490 tiles
NP = NTP // 2               # 245 tile pairs
NPAD = NTP * PT             # 62720 padded points per core
DA = D + 1                  # 65: X augmented with ones column
DH = D + 3                  # 67: xh rows + three c2 ones rows
XTF = TPS * PT              # 896 columns of x^T per slab
KC = K // PT                # 4 k-chunks for the transposed mm2
NTPE = -(-NS // PT)         # 489 tiles actually containing real points
                            # (tile 489 of the padded layout is pure pad)

_CACHE: dict = {}


def _build_nc():
    from contextlib import ExitStack

    import concourse.bacc as bacc
    import concourse.tile as tile
    from concourse import mybir

    f32 = mybir.dt.float32
    bf16 = mybir.dt.bfloat16
    f16 = mybir.dt.float16

    nc = bacc.Bacc("TRN2", target_bir_lowering=False, debug=False)

    # boot: cha | clb | xall tile0+1 | xht tile0+1  (one DMA)
    BOOTW = 2 * K + 4 * PT
    boot = nc.dram_tensor("boot", [PT, BOOTW], bf16, kind="ExternalInput")
    xall = nc.dram_tensor("xall", [PT, NSLAB, XTF], bf16, kind="ExternalInput")
    xht = nc.dram_tensor("xht", [DH, NSLAB, XTF], bf16, kind="ExternalInput")
    xa = nc.dram_tensor("xa", [PT, NSLAB, TPS * DA], f16, kind="ExternalInput")
    sout = nc.dram_tensor("sout", [PT, KC, DA], f32, kind="ExternalOutput")

    with tile.TileContext(nc) as tc, ExitStack() as ctx:
        const = ctx.enter_context(tc.tile_pool(name="const", bufs=1))
        ld = ctx.enter_context(tc.tile_pool(name="ld", bufs=1))
        rp = ctx.enter_context(tc.tile_pool(name="r", bufs=1))
        mred = ctx.enter_context(tc.tile_pool(name="mred", bufs=1))
        gp = ctx.enter_context(tc.tile_pool(name="g", bufs=1, space="PSUM"))
        sp = ctx.enter_context(tc.tile_pool(name="s", bufs=1, space="PSUM"))

        boot_sb = const.tile([PT, BOOTW], bf16)
        nc.sync.dma_start(boot_sb[:, 0 : K + 2 * PT], boot[:, 0 : K + 2 * PT])
        nc.sync.dma_start(
            boot_sb[:, K + 2 * PT :], boot[:, K + 2 * PT :]
        )
        cha = boot_sb[:, 0:K]                     # [ch; ch]
        clb = boot_sb[:DH, K + 2 * PT : 2 * K + 2 * PT]  # [cl; c2a; c2b; c2c]

        # S'^T accumulator: [k-row, k-chunk, d] in one PSUM bank,
        # chunks padded to 512B so each matmul output is 512-byte aligned.
        # The bank is zeroed ONCE up front (a matmul's start=True would zero
        # the whole 2KB region, wiping sibling chunks), and every mm2
        # accumulates with start=False + skip_group_check.
        s_ps = sp.tile([PT, KC, PT], f32)
        nc.vector.memset(s_ps[:], 0.0)

        # Manual tile rings (instead of per-iteration pool allocs): Tile
        # emits a release-event pair per allocated tile, and those events
        # serialize the engine sequencers; fixed tiles keep the WAR/RAW
        # tracking without the release machinery.
        QB, MB, RGB, LB = 5, 6, 4, 4
        q_ring = [
            gp.tile([PT, K], f32, name=f"q{i}", tag=f"q{i}") for i in range(QB)
        ]
        m_ring = [
            mred.tile([PT, 1], f32, name=f"m{i}", tag=f"m{i}") for i in range(MB)
        ]
        r_ring = [
            rp.tile([PT, K], f16, name=f"r{i}", tag=f"r{i}") for i in range(RGB)
        ]
        ld_ring = [
            (
                ld.tile([PT, XTF], bf16, name=f"xall{i}", tag=f"xall{i}"),
                ld.tile([DH, XTF], bf16, name=f"xht{i}", tag=f"xht{i}"),
                ld.tile([PT, TPS * DA], f16, name=f"xa{i}", tag=f"xa{i}"),
            )
            for i in range(LB)
        ]
        slabs = [None] * NSLAB
        PF = 1  # slab DMA lookahead (in slabs)

        def emit_dma(si):
            xall_t, xht_t, xa_t = ld_ring[si % LB]
            nc.sync.dma_start(xall_t[:], xall[:, si, :])
            nc.gpsimd.dma_start(xht_t[:], xht[:, si, :])
            nc.gpsimd.dma_start(xa_t[:], xa[:, si, :])
            slabs[si] = (xall_t, xht_t, xa_t)

        def emit_mm1(g):
            si, tt = divmod(g, TPS)
            xall_t, xht_t, _ = slabs[si]
            if g < 2:
                # bootstrap: tiles 0-1 lhsT ride the boot DMA so the first
                # matmuls start right after one small transfer
                lhs_a = boot_sb[:, K + g * PT : K + (g + 1) * PT]
                lhs_b = boot_sb[:DH, 2 * K + (2 + g) * PT : 2 * K + (3 + g) * PT]
            else:
                lhs_a = xall_t[:, tt * PT : (tt + 1) * PT]
                lhs_b = xht_t[:, tt * PT : (tt + 1) * PT]
            q_ps = q_ring[g % QB]
            nc.tensor.matmul(q_ps[:], lhs_a, cha, start=True, stop=False)
            nc.tensor.matmul(q_ps[:], lhs_b, clb, start=False, stop=True)

        def emit_reduce(g):
            nc.vector.tensor_reduce(
                out=m_ring[g % MB][:],
                in_=q_ring[g % QB][:],
                axis=mybir.AxisListType.X,
                op=mybir.AluOpType.min,
                negate=True,
            )

        def emit_sign(g):
            nc.scalar.activation(
                out=r_ring[g % RGB][:],
                in_=q_ring[g % QB][:],
                func=mybir.ActivationFunctionType.Sign,
                bias=m_ring[g % MB][:],
                scale=1.0,
            )

        def emit_mm2(g, first, last):
            si, tt = divmod(g, TPS)
            _, _, xa_t = slabs[si]
            xa_rhs = xa_t[:, tt * DA : (tt + 1) * DA]
            r_t = r_ring[g % RGB]
            for kc in range(KC):
                nc.tensor.matmul(
                    s_ps[:, kc, 0:DA],
                    r_t[:, kc * PT : (kc + 1) * PT],
                    xa_rhs,
                    start=False,
                    stop=False,
                    skip_group_check=True,
                )

        for g in range(NTPE + 4):
            if g < NTPE:
                si, tt = divmod(g, TPS)
                if g == 0:
                    for s0 in range(PF + 1):
                        emit_dma(s0)
                elif tt == 0 and si + PF < NSLAB:
                    emit_dma(si + PF)
                emit_mm1(g)
            if 0 <= g - 1 < NTPE:
                emit_reduce(g - 1)
            if 0 <= g - 2 < NTPE:
                emit_sign(g - 2)
            if 0 <= g - 3 < NTPE:
                emit_mm2(g - 3, first=(g - 3 == 0), last=(g - 3 == NTPE - 1))

        s_sb = const.tile([PT, KC, DA], f32)
        nc.vector.tensor_copy(s_sb[:], s_ps[:, :, 0:DA])
        nc.sync.dma_start(sout[:], s_sb[:])

    nc.compile()
    return nc


def _get_nc():
    if "nc" not in _CACHE:
        _CACHE["nc"] = _build_nc()
    return _CACHE["nc"]


def build_in_maps(X, idx):
    import ml_dtypes

    bf = ml_dtypes.bfloat16

    C = X[idx].astype(np.float64)  # [K, D]
    c2h = 0.5 * np.einsum("kd,kd->k", C, C)

    cb = -C.T  # [D, K] float64
    ch = cb.astype(bf)
    cl = (cb - ch.astype(np.float64)).astype(bf)
    c2a = c2h.astype(bf)
    c2b = (c2h - c2a.astype(np.float64)).astype(bf)
    c2c = (c2h - c2a.astype(np.float64) - c2b.astype(np.float64)).astype(bf)

    cha_np = np.concatenate([ch, ch], axis=0)                    # [128, K]
    clb_np = np.concatenate([cl, c2a[None], c2b[None], c2c[None]], axis=0)  # [67, K]

    in_maps = []
    for c in range(NCORES):
        xs = X[c * NS : (c + 1) * NS]  # [NS, D] float32
        xh = xs.astype(bf)
        xl = (xs - xh.astype(np.float32)).astype(bf)

        xall_np = np.zeros((PT, NPAD), bf)
        xall_np[:D, :NS] = xh.T
        xall_np[D:, :NS] = xl.T
        xht_np = np.zeros((DH, NPAD), bf)
        xht_np[:D, :NS] = xh.T
        xht_np[D:, :NS] = 1.0

        xa_np = np.zeros((NPAD, DA), np.float16)
        xa_np[:NS, :D] = xs.astype(np.float16)
        xa_np[:NS, D] = 1.0
        xa_tiled = np.ascontiguousarray(
            xa_np.reshape(NTP, PT, DA).transpose(1, 0, 2)
        ).reshape(PT, NSLAB, TPS * DA)

        boot_np = np.zeros((PT, BOOTW_H), bf)
        boot_np[:, :K] = cha_np
        boot_np[:, K : K + 2 * PT] = xall_np[:, : 2 * PT]
        boot_np[:DH, K + 2 * PT : 2 * K + 2 * PT] = clb_np
        boot_np[:DH, 2 * K + 2 * PT :] = xht_np[:, : 2 * PT]

        in_maps.append(
            {
                "boot": boot_np,
                "xall": np.ascontiguousarray(xall_np.reshape(PT, NSLAB, XTF)),
                "xht": np.ascontiguousarray(xht_np.reshape(DH, NSLAB, XTF)),
                "xa": xa_tiled,
            }
        )
    return in_maps


BOOTW_H = 2 * K + 4 * PT


def kernel(X, init_idx):
    from concourse.bass_utils import run_bass_kernel_spmd

    X = np.ascontiguousarray(np.asarray(X, dtype=np.float32))
    idx = np.asarray(init_idx).astype(np.int64)

    in_maps = build_in_maps(X, idx)
    _CACHE["in_maps"] = in_maps

    # Build a fresh Bass module per call: executing via run_bass_kernel_spmd
    # mutates the module, and re-running a previously-executed one crashes
    # the device (NRT_EXEC_UNIT_UNRECOVERABLE).
    nc = _build_nc()
    res = run_bass_kernel_spmd(nc, in_maps, core_ids=list(range(NCORES)))

    SpT = np.zeros((PT, KC, DA), np.float64)
    for r in res.results:
        SpT += r["sout"].astype(np.float64)
    # S'[d, k] with k = kc*128 + kr  <-  SpT[kr, kc, d]
    Sp = np.transpose(SpT, (2, 1, 0)).reshape(DA, K)

    colsum = X.astype(np.float16).astype(np.float64).sum(axis=0)  # [D]
    sums = colsum[:, None] - Sp[:D]                # [D, K]
    counts = float(N) - Sp[D]                      # [K]
    out = (sums / np.maximum(counts, 1.0)[None, :]).T.astype(np.float32)
    return out


# revision 38
# speedup vs baseline: 1.1110x; 1.0012x over previous
"""MiniBatch K-means (1 iteration) on 8 Trainium2 NeuronCores.

Strategy (data-parallel over points, per sharding hint):
  - Shard X along N across 8 cores (62500 points each, zero-padded to
    62720 = 490 tiles of 128 points; the last tile is pure padding and
    its compute is skipped entirely -> 489 pipeline iterations).
  - Per 128-point tile on each core (steady state ~678ns/tile, DVE-bound
    with zero steady-state gaps):
      mm1 (PE, 2x213ns): q[n,k] = c2[k]/2 - x_n.c_k via the bf16 hi/lo
            error-compensated pair of matmuls (x = xh+xl, c ~ ch+cl):
              mm1a: [xh;xl] @ [ch;ch]   (128-row contraction)
              mm1b: [xh;1,1,1] @ [cl; c2a; c2b; c2c]  (67-row)
            Distance error ~2^-18: assignments match the f32 reference
            to a handful of boundary flips. (f32r matmuls are TF32-
            precision on this stack, so full-rate exact fp32 does not
            exist; fp8 3-piece schemes flip too many assignments.)
      DVE (658ns): tensor_reduce(min, negate) over the [128, 512] PSUM
            tile -> mneg = -min_k q. This is the bottleneck and it is
            irreducible: DVE is the only free-axis reducer, the ISA
            allows only one PSUM-read operand per instruction (so no
            tensor_tensor_reduce folding), GPSIMD cannot touch PSUM,
            and batching tiles per reduce makes the Activation engine
            self-serialize on its own tick semaphore (+219ns/pair,
            a worse trade).
      ACT (612ns): r[n,k] = sign(q + mneg) -> EXACT complement
            indicator (0 at the argmin, 1 elsewhere) as f16 {0,1}.
            No exp/beta/leakage; ties double-count, which the metric
            tolerates. Gated per tile by its own reduce, so every
            engine is cross-gated just-in-time and no Tile semaphore
            tax accrues.
      mm2 (PE, 4x27ns): TRANSPOSED sums: for each 128-wide k-chunk kc,
              S_T[:, kc, :] += r[:, kc*128:(kc+1)*128].T @ [X|1]_tile
            lhsT = the indicator chunk (stationary), rhs = the f16
            point-major [128, 65] tile, so each matmul streams only 65
            columns; counts ride along as the ones column of the rhs.
            All four chunk accumulators live in ONE PSUM bank, pre-
            zeroed once (a matmul start=True would zero the whole 2KB
            region and wipe sibling chunks), accumulating with
            start=False + skip_group_check.
  - Host: S^T chunks -> S' [65, 512]; S = colsum(f16(X)) - sum_cores S',
    counts = N - S'[64], divide, transpose.

The complement convention exists because the Activation engine can only
produce an exact indicator via Sign (sign(q-m) is 0 at the argmin, 1
elsewhere); the host subtracts per-core column totals of the SAME f16
rounding the device sums. Padded points have all-zero [X|1] rows.

Startup/teardown: the first boot DMA carries only mm1a's operands
(cha + tiles 0/1 of xall) so the first matmul issues ~3.2us in; clb and
the xht bootstrap ride a second DMA. xht/xa stream over the SWDGE
(gpsimd) queue to keep the HWDGE serial chain short. Engine busy:
DVE 97%, ACT ~90%, PE ~79% of the 331.7us total.
"""

import numpy as np

N, D, K = 500000, 64, 512
NCORES = 8
NS = N // NCORES            # 62500 points per core
PT = 128                    # points per tile (partition dim)
TPS = 7                     # tiles per DMA slab
NSLAB = -(-NS // (PT * TPS))  # 70 slabs
NTP = NSLAB * TPS           # 490 tiles
NP = NTP // 2               # 245 tile pairs
NPAD = NTP * PT             # 62720 padded points per core
DA = D + 1                  # 65: X augmented with ones column
DH = D + 3                  # 67: xh rows + three c2 ones rows
XTF = TPS * PT              # 896 columns of x^T per slab
KC = K // PT                # 4 k-chunks for the transposed mm2
NTPE = -(-NS // PT)         # 489 tiles actually containing real points
                            # (tile 489 of the padded layout is pure pad)

_CACHE: dict = {}


def _build_nc():
    from contextlib import ExitStack

    import concourse.bacc as bacc
    import concourse.tile as tile
    from concourse import mybir

    f32 = mybir.dt.float32
    bf16 = mybir.dt.bfloat16
    f16 = mybir.dt.float16

    nc = bacc.Bacc("TRN2", target_bir_lowering=False, debug=False)

    # boot: cha | clb | xall tile0+1 | xht tile0+1  (one DMA)
    BOOTW = 2 * K + 4 * PT
    boot = nc.dram_tensor("boot", [PT, BOOTW], bf16, kind="ExternalInput")
    xall = nc.dram_tensor("xall", [PT, NSLAB, XTF], bf16, kind="ExternalInput")
    xht = nc.dram_tensor("xht", [DH, NSLAB, XTF], bf16, kind="ExternalInput")
    xa = nc.dram_tensor("xa", [PT, NSLAB, TPS * DA], f16, kind="ExternalInput")
    sout = nc.dram_tensor("sout", [PT, KC, DA], f32, kind="ExternalOutput")

    with tile.TileContext(nc) as tc, ExitStack() as ctx:
        const = ctx.enter_context(tc.tile_pool(name="const", bufs=1))
        ld = ctx.enter_context(tc.tile_pool(name="ld", bufs=1))
        rp = ctx.enter_context(tc.tile_pool(name="r", bufs=1))
        mred = ctx.enter_context(tc.tile_pool(name="mred", bufs=1))
        gp = ctx.enter_context(tc.tile_pool(name="g", bufs=1, space="PSUM"))
        sp = ctx.enter_context(tc.tile_pool(name="s", bufs=1, space="PSUM"))

        boot_sb = const.tile([PT, BOOTW], bf16)
        nc.sync.dma_start(boot_sb[:, 0 : K + 2 * PT], boot[:, 0 : K + 2 * PT])
        # boot-2 (clb + xht bootstrap) goes out on the SWDGE queue FIRST,
        # before the slab desc-gens, overlapping boot-1's HWDGE path
        nc.gpsimd.dma_start(boot_sb[:, K + 2 * PT :], boot[:, K + 2 * PT :])
        cha = boot_sb[:, 0:K]                     # [ch; ch]
        clb = boot_sb[:DH, K + 2 * PT : 2 * K + 2 * PT]  # [cl; c2a; c2b; c2c]

        # S'^T accumulator: [k-row, k-chunk, d] in one PSUM bank,
        # chunks padded to 512B so each matmul output is 512-byte aligned.
        # The bank is zeroed ONCE up front (a matmul's start=True would zero
        # the whole 2KB region, wiping sibling chunks), and every mm2
        # accumulates with start=False + skip_group_check.
        s_ps = sp.tile([PT, KC, PT], f32)
        nc.vector.memset(s_ps[:], 0.0)

        # Manual tile rings (instead of per-iteration pool allocs): Tile
        # emits a release-event pair per allocated tile, and those events
        # serialize the engine sequencers; fixed tiles keep the WAR/RAW
        # tracking without the release machinery.
        QB, MB, RGB, LB = 5, 6, 4, 4
        q_ring = [
            gp.tile([PT, K], f32, name=f"q{i}", tag=f"q{i}") for i in range(QB)
        ]
        m_ring = [
            mred.tile([PT, 1], f32, name=f"m{i}", tag=f"m{i}") for i in range(MB)
        ]
        r_ring = [
            rp.tile([PT, K], f16, name=f"r{i}", tag=f"r{i}") for i in range(RGB)
        ]
        ld_ring = [
            (
                ld.tile([PT, XTF], bf16, name=f"xall{i}", tag=f"xall{i}"),
                ld.tile([DH, XTF], bf16, name=f"xht{i}", tag=f"xht{i}"),
                ld.tile([PT, TPS * DA], f16, name=f"xa{i}", tag=f"xa{i}"),
            )
            for i in range(LB)
        ]
        slabs = [None] * NSLAB
        PF = 1  # slab DMA lookahead (in slabs)

        def emit_dma(si):
            xall_t, xht_t, xa_t = ld_ring[si % LB]
            nc.sync.dma_start(xall_t[:], xall[:, si, :])
            nc.gpsimd.dma_start(xht_t[:], xht[:, si, :])
            nc.gpsimd.dma_start(xa_t[:], xa[:, si, :])
            slabs[si] = (xall_t, xht_t, xa_t)

        def emit_mm1(g):
            si, tt = divmod(g, TPS)
            xall_t, xht_t, _ = slabs[si]
            if g < 2:
                # bootstrap: tiles 0-1 lhsT ride the boot DMA so the first
                # matmuls start right after one small transfer
                lhs_a = boot_sb[:, K + g * PT : K + (g + 1) * PT]
                lhs_b = boot_sb[:DH, 2 * K + (2 + g) * PT : 2 * K + (3 + g) * PT]
            else:
                lhs_a = xall_t[:, tt * PT : (tt + 1) * PT]
                lhs_b = xht_t[:, tt * PT : (tt + 1) * PT]
            q_ps = q_ring[g % QB]
            nc.tensor.matmul(q_ps[:], lhs_a, cha, start=True, stop=False)
            nc.tensor.matmul(q_ps[:], lhs_b, clb, start=False, stop=True)

        def emit_reduce(g):
            nc.vector.tensor_reduce(
                out=m_ring[g % MB][:],
                in_=q_ring[g % QB][:],
                axis=mybir.AxisListType.X,
                op=mybir.AluOpType.min,
                negate=True,
            )

        def emit_sign(g):
            nc.scalar.activation(
                out=r_ring[g % RGB][:],
                in_=q_ring[g % QB][:],
                func=mybir.ActivationFunctionType.Sign,
                bias=m_ring[g % MB][:],
                scale=1.0,
            )

        def emit_mm2(g, first, last):
            si, tt = divmod(g, TPS)
            _, _, xa_t = slabs[si]
            xa_rhs = xa_t[:, tt * DA : (tt + 1) * DA]
            r_t = r_ring[g % RGB]
            for kc in range(KC):
                nc.tensor.matmul(
                    s_ps[:, kc, 0:DA],
                    r_t[:, kc * PT : (kc + 1) * PT],
                    xa_rhs,
                    start=False,
                    stop=False,
                    skip_group_check=True,
                )

        for g in range(NTPE + 4):
            if g < NTPE:
                si, tt = divmod(g, TPS)
                if g == 0:
                    for s0 in range(PF + 1):
                        emit_dma(s0)
                elif tt == 0 and si + PF < NSLAB:
                    emit_dma(si + PF)
                emit_mm1(g)
            if 0 <= g - 1 < NTPE:
                emit_reduce(g - 1)
            if 0 <= g - 2 < NTPE:
                emit_sign(g - 2)
            if 0 <= g - 3 < NTPE:
                emit_mm2(g - 3, first=(g - 3 == 0), last=(g - 3 == NTPE - 1))

        s_sb = const.tile([PT, KC, DA], f32)
        nc.vector.tensor_copy(s_sb[:], s_ps[:, :, 0:DA])
        nc.sync.dma_start(sout[:], s_sb[:])

    nc.compile()
    return nc


def _get_nc():
    if "nc" not in _CACHE:
        _CACHE["nc"] = _build_nc()
    return _CACHE["nc"]


def build_in_maps(X, idx):
    import ml_dtypes

    bf = ml_dtypes.bfloat16

    C = X[idx].astype(np.float64)  # [K, D]
    c2h = 0.5 * np.einsum("kd,kd->k", C, C)

    cb = -C.T  # [D, K] float64
    ch = cb.astype(bf)
    cl = (cb - ch.astype(np.float64)).astype(bf)
    c2a = c2h.astype(bf)
    c2b = (c2h - c2a.astype(np.float64)).astype(bf)
    c2c = (c2h - c2a.astype(np.float64) - c2b.astype(np.float64)).astype(bf)

    cha_np = np.concatenate([ch, ch], axis=0)                    # [128, K]
    clb_np = np.concatenate([cl, c2a[None], c2b[None], c2c[None]], axis=0)  # [67, K]

    in_maps = []
    for c in range(NCORES):
        xs = X[c * NS : (c + 1) * NS]  # [NS, D] float32
        xh = xs.astype(bf)
        xl = (xs - xh.astype(np.float32)).astype(bf)

        xall_np = np.zeros((PT, NPAD), bf)
        xall_np[:D, :NS] = xh.T
        xall_np[D:, :NS] = xl.T
        xht_np = np.zeros((DH, NPAD), bf)
        xht_np[:D, :NS] = xh.T
        xht_np[D:, :NS] = 1.0

        xa_np = np.zeros((NPAD, DA), np.float16)
        xa_np[:NS, :D] = xs.astype(np.float16)
        xa_np[:NS, D] = 1.0
        xa_tiled = np.ascontiguousarray(
            xa_np.reshape(NTP, PT, DA).transpose(1, 0, 2)
        ).reshape(PT, NSLAB, TPS * DA)

        boot_np = np.zeros((PT, BOOTW_H), bf)
        boot_np[:, :K] = cha_np
        boot_np[:, K : K + 2 * PT] = xall_np[:, : 2 * PT]
        boot_np[:DH, K + 2 * PT : 2 * K + 2 * PT] = clb_np
        boot_np[:DH, 2 * K + 2 * PT :] = xht_np[:, : 2 * PT]

        in_maps.append(
            {
                "boot": boot_np,
                "xall": np.ascontiguousarray(xall_np.reshape(PT, NSLAB, XTF)),
                "xht": np.ascontiguousarray(xht_np.reshape(DH, NSLAB, XTF)),
                "xa": xa_tiled,
            }
        )
    return in_maps


BOOTW_H = 2 * K + 4 * PT


def kernel(X, init_idx):
    from concourse.bass_utils import run_bass_kernel_spmd

    X = np.ascontiguousarray(np.asarray(X, dtype=np.float32))
    idx = np.asarray(init_idx).astype(np.int64)

    in_maps = build_in_maps(X, idx)
    _CACHE["in_maps"] = in_maps

    # Build a fresh Bass module per call: executing via run_bass_kernel_spmd
    # mutates the module, and re-running a previously-executed one crashes
    # the device (NRT_EXEC_UNIT_UNRECOVERABLE).
    nc = _build_nc()
    res = run_bass_kernel_spmd(nc, in_maps, core_ids=list(range(NCORES)))

    SpT = np.zeros((PT, KC, DA), np.float64)
    for r in res.results:
        SpT += r["sout"].astype(np.float64)
    # S'[d, k] with k = kc*128 + kr  <-  SpT[kr, kc, d]
    Sp = np.transpose(SpT, (2, 1, 0)).reshape(DA, K)

    colsum = X.astype(np.float16).astype(np.float64).sum(axis=0)  # [D]
    sums = colsum[:, None] - Sp[:D]                # [D, K]
    counts = float(N) - Sp[D]                      # [K]
    out = (sums / np.maximum(counts, 1.0)[None, :]).T.astype(np.float32)
    return out
